# revision 42
# baseline (speedup 1.0000x reference)
"""MeshPoolFace segment-mean pooling kernel for Trainium2 (8 NeuronCores).

Problem: fe [B=16, C=256, F=16000] f32, group_ids [B, F] int in [0, 8000)
Output: [B, C, 8000] f32 = per-(mesh,channel) segment mean of face features.

Data-parallel over B (2 meshes per core). fe is converted to bf16 on the
host (~0.4% quantization, well under the 2e-2 gate); the device writes the
output bf16 in [T, C] layout and the host transposes/upcasts.

Per mesh, on device:
  phase A: bucket = gid // 250 (32 uniform buckets x 250 targets). Compute
    a unique, collision-free scatter slot per face: slot = bucket*CAP +
    rank-within-bucket via matmul prefix sums (strict-lower-triangular
    matmuls). Collision-free slots make the scatter's CCE add an exact
    row write (no read-modify-write races between SDMA engines).
  fwd: stream fe face-major via the DMA xbar transpose (no PE work),
    assemble rows [256 data | 1.0 | gl+1 | pad] (bf16, 260 payload) and
    dma_scatter_add them into the binned DRAM table [32*CAP, 384]
    (row stride 384 elems = 768 B, multiple of 256 B as SWDGE requires;
    only 260 elems per row are written/read). Padding slots (rows
    [416, CAP) of each bucket; every bucket count is >= 443 for this
    input distribution) are pre-zeroed by 4 strided broadcast DMAs.
  phase 2: per bucket, load its CAP rows (260-wide), build one-hot
    (gl+1 vs iota 1..256) in bf16, matmul-accumulate sums+counts in
    PSUM, divide via Scalar-engine per-partition scale, and DMA the
    [targets, C] rows straight out (no transpose on device).
"""

import sys

sys.path.insert(0, "/opt/trn_rl_repo")

import numpy as np

B, C, F, T = 16, 256, 16000, 8000
N_CORES = 8
MPC = B // N_CORES  # meshes per core

P = 128
NBUK = 32            # buckets (gid // 250)
TPB = 250            # targets per bucket (uniform: 32*250 = 8000)
CAP = 640            # slots per bucket (max observed count 562)
NCH = CAP // P       # 5 chunks per bucket
SLOTS = NBUK * CAP   # 20480
EW = 384             # table row stride (bf16 elems; 768 B = 3*256 B)
SRC_W = 260          # scattered payload: 256 data | 1.0 | gl+1 | 2 pad
BW = 256             # one-hot width (gl+1 in [1,250])
# (full-table zeroing; scatter is +=)

NBATCH = 5
CHUNKS = F // P            # 125
BCHUNK = CHUNKS // NBATCH  # 25 chunks per scatter batch
BF = BCHUNK * P            # 3200 faces per batch
COLS = F // P              # 125 (phase-A compute layout columns)


def build_nc():
    import concourse.bacc as bacc
    import concourse.bass as bass
    import concourse.tile as tile
    from concourse import library_config, mybir
    from concourse.masks import make_identity

    f32 = mybir.dt.float32
    bf16 = mybir.dt.bfloat16
    i32 = mybir.dt.int32
    i16 = mybir.dt.int16
    AL = mybir.AluOpType

    nc = bacc.Bacc("TRN2", debug=False)

    fe_d = nc.dram_tensor("fe", [MPC * C, F], bf16, kind="ExternalInput")
    gid_d = nc.dram_tensor("gid", [MPC, F], i32, kind="ExternalInput")
    out_d = nc.dram_tensor("out", [MPC * T + 8, C], bf16,
                           kind="ExternalOutput")
    tbl_ds = [
        nc.dram_tensor(f"tbl{m}", [SLOTS, EW], bf16) for m in range(MPC)
    ]
    slot_d = nc.dram_tensor("slot_s", [MPC, F], f32)
    gl_d = nc.dram_tensor("gl_s", [MPC, F], f32)
    b2_d = nc.dram_tensor("b2_s", [MPC, COLS, NBUK], f32)


    with tile.TileContext(nc) as tc:
        with (
            tc.tile_pool(name="singles", bufs=1) as singles,
            tc.tile_pool(name="masks", bufs=1) as mask_pool,
            tc.tile_pool(name="b2p", bufs=1) as b2_pool,
            tc.tile_pool(name="small", bufs=4) as small_pool,
            tc.tile_pool(name="src", bufs=4) as src_pool,
            tc.tile_pool(name="tbp", bufs=4) as tb_pool,
            tc.tile_pool(name="p2", bufs=6) as p2_pool,
            tc.tile_pool(name="psum", bufs=1, space="PSUM") as psum_pool,
        ):
            nc.gpsimd.load_library(library_config.mlp)

            # ---------- constants ----------
            ident = singles.tile([P, P], f32)
            make_identity(nc, ident[:])
            identb = singles.tile([P, P], bf16)
            nc.vector.tensor_copy(out=identb[:], in_=ident[:])

            ones_col = singles.tile([P, 1], f32)
            nc.vector.memset(ones_col[:], 1.0)
            ones_row = singles.tile([1, COLS], f32)
            nc.vector.memset(ones_row[:], 1.0)

            # strict lower triangular: L[p, x] = 1 iff p < x
            Ls = singles.tile([P, P], f32)
            nc.gpsimd.memset(Ls[:], 0.0)
            nc.gpsimd.affine_select(
                out=Ls[:], in_=Ls[:], pattern=[[-1, P]],
                compare_op=AL.is_ge, fill=1.0, base=0, channel_multiplier=1,
            )
            # augmented [COLS+1, COLS]: strict-lower + all-ones last row
            La = singles.tile([COLS + 1, COLS], f32)
            nc.gpsimd.memset(La[:], 0.0)
            nc.gpsimd.affine_select(
                out=La[:], in_=La[:], pattern=[[-1, COLS]],
                compare_op=AL.is_ge, fill=1.0, base=0, channel_multiplier=1,
            )
            nc.sync.dma_start(out=La[COLS : COLS + 1, :], in_=ones_row[:])

            # iota row 1..256 (bf16, exact) for one-hot compare
            io32 = singles.tile([P, BW], i32)
            nc.gpsimd.iota(io32[:], pattern=[[1, BW]], base=1,
                           channel_multiplier=0)
            iob = singles.tile([P, BW], bf16)
            nc.vector.tensor_copy(out=iob[:], in_=io32[:])

            # row of bucket bases b*CAP
            bc32 = singles.tile([1, NBUK], i32)
            nc.gpsimd.iota(bc32[:], pattern=[[CAP, NBUK]], base=0,
                           channel_multiplier=0)
            bcap = singles.tile([1, NBUK], f32)
            nc.vector.tensor_copy(out=bcap[:], in_=bc32[:])

            zeros = singles.tile([P, 4096], bf16)
            nc.gpsimd.memset(zeros[:], 0.0)

            INV = float(np.float32(1.0) / np.float32(TPB))

            glrs, ix16 = [], []

            # ---------- phase A ----------
            def phaseA(m):
                g32 = small_pool.tile([P, COLS], i32, tag="g32",
                                      name=f"g32_{m}")
                nc.sync.dma_start(out=g32[:], in_=gid_d[m, :].rearrange(
                    "(p c) -> p c", p=P))
                gf = small_pool.tile([P, COLS], f32, tag="gf",
                                     name=f"gf_{m}")
                nc.vector.tensor_copy(out=gf[:], in_=g32[:])
                # bucket = floor(g/250), robust to either f32->int rounding
                xb = small_pool.tile([P, COLS], f32, tag="xb")
                nc.vector.tensor_scalar(xb[:], gf[:], INV, None, AL.mult)
                y32 = small_pool.tile([P, COLS], i32, tag="y32")
                nc.vector.tensor_copy(out=y32[:], in_=xb[:])
                bkf = small_pool.tile([P, COLS], f32, tag="bkf",
                                      name=f"bkf_{m}")
                nc.vector.tensor_copy(out=bkf[:], in_=y32[:])
                over = small_pool.tile([P, COLS], f32, tag="over")
                nc.vector.tensor_tensor(over[:], bkf[:], xb[:], AL.is_gt)
                nc.vector.tensor_tensor(bkf[:], bkf[:], over[:], AL.subtract)
                # gl+1 = g - 250*bucket + 1
                glf = small_pool.tile([P, COLS], f32, tag="glf",
                                      name=f"glf_{m}")
                nc.vector.tensor_scalar(glf[:], bkf[:], float(-TPB), 1.0,
                                        AL.mult, AL.add)
                nc.vector.tensor_tensor(glf[:], glf[:], gf[:], AL.add)
                # gl+1 to DRAM (face-order f = p*COLS + c)
                nc.scalar.dma_start(
                    out=bass.AP(tensor=gl_d, offset=m * F,
                                ap=[[COLS, P], [1, COLS]]),
                    in_=glf[:],
                )

                # bucket masks
                M = mask_pool.tile([P, NBUK, COLS], f32, tag="M")
                for b in range(NBUK):
                    nc.vector.tensor_scalar(M[:, b, :], bkf[:], float(b),
                                            None, AL.is_equal)

                # per-column bucket counts -> [COLS, NBUK] psum
                cnt_ps = psum_pool.tile([COLS, NBUK], f32, tag="psA", bufs=2)
                for b in range(NBUK):
                    nc.tensor.matmul(cnt_ps[:, b : b + 1], M[:, b, :],
                                     ones_col[:], start=True, stop=True)
                cnt_aug = small_pool.tile([COLS + 1, NBUK], f32, tag="caug")
                nc.vector.tensor_copy(out=cnt_aug[0:COLS, :], in_=cnt_ps[:])
                nc.sync.dma_start(out=cnt_aug[COLS : COLS + 1, :],
                                  in_=bcap[:])
                # base2'[c, b] = sum_{c'<c} cnt[c', b] + b*CAP
                b2_ps = psum_pool.tile([COLS, NBUK], f32, tag="psA", bufs=2)
                nc.tensor.matmul(b2_ps[:], La[:], cnt_aug[:],
                                 start=True, stop=True)
                b2_sb = small_pool.tile([COLS, NBUK], f32, tag="b2sb")
                nc.vector.tensor_copy(out=b2_sb[:], in_=b2_ps[:])
                nc.sync.dma_start(out=b2_d[m, :, :], in_=b2_sb[:])
                # broadcast bases to all partitions
                B2 = b2_pool.tile([P, COLS * NBUK], f32, tag="B2")
                nc.scalar.dma_start(
                    out=B2[:],
                    in_=bass.AP(tensor=b2_d, offset=m * COLS * NBUK,
                                ap=[[0, P], [1, COLS * NBUK]]),
                )
                B2v = B2[:].rearrange("p (c b) -> p c b", b=NBUK)

                # slot[p,c] = rank-within-bucket + base2'[c,b] for own bucket
                NACC = 4
                accs = [
                    small_pool.tile([P, COLS], f32, tag=f"acc{a}",
                                    name=f"acc{m}_{a}")
                    for a in range(NACC)
                ]
                NQ = 4  # buckets per cs matmul
                for q in range(NBUK // NQ):
                    cs_ps = psum_pool.tile([P, NQ * COLS], f32, tag="psB",
                                           bufs=1, name=f"cs{m}_{q}")
                    nc.tensor.matmul(
                        cs_ps[:],
                        Ls[:],
                        M[:, q * NQ : (q + 1) * NQ, :].rearrange(
                            "p b c -> p (b c)"),
                        start=True, stop=True,
                    )
                    cs_sb = small_pool.tile([P, NQ * COLS], f32, tag="cssb",
                                            name=f"cssb{m}_{q}")
                    nc.scalar.copy(out=cs_sb[:], in_=cs_ps[:])
                    for bi in range(NQ):
                        b = q * NQ + bi
                        eng = nc.vector
                        acc = accs[(b % 2) * 2 + (b // 2) % 2]
                        t1 = small_pool.tile([P, COLS], f32, tag="t1",
                                             name=f"t1_{m}_{b}")
                        eng.tensor_tensor(t1[:],
                                          cs_sb[:, bi * COLS : (bi + 1) * COLS],
                                          B2v[:, :, b], AL.add)
                        if b < NACC:
                            eng.tensor_tensor(acc[:], t1[:], M[:, b, :],
                                              AL.mult)
                        else:
                            t2 = small_pool.tile([P, COLS], f32, tag="t2",
                                                 name=f"t2_{m}_{b}")
                            eng.tensor_tensor(t2[:], t1[:], M[:, b, :],
                                              AL.mult)
                            eng.tensor_tensor(acc[:], acc[:], t2[:], AL.add)
                slotf = small_pool.tile([P, COLS], f32, tag="slotf")
                nc.vector.tensor_tensor(accs[0][:], accs[0][:], accs[2][:],
                                        AL.add)
                nc.vector.tensor_tensor(accs[1][:], accs[1][:], accs[3][:],
                                        AL.add)
                nc.vector.tensor_tensor(slotf[:], accs[0][:], accs[1][:],
                                        AL.add)
                nc.sync.dma_start(
                    out=bass.AP(tensor=slot_d, offset=m * F,
                                ap=[[COLS, P], [1, COLS]]),
                    in_=slotf[:],
                )

                # reload slot/gl as [125, 128] (contiguous, f = 128a + x);
                # gl: PE-transpose to face-chunk layout [x, a]
                m2s = small_pool.tile([COLS, P], f32, tag="m2s")
                nc.scalar.dma_start(
                    out=m2s[:],
                    in_=slot_d[m, :].rearrange("(a x) -> a x", a=COLS),
                )
                m2g = small_pool.tile([COLS, P], f32, tag="m2g")
                nc.scalar.dma_start(
                    out=m2g[:],
                    in_=gl_d[m, :].rearrange("(a x) -> a x", a=COLS),
                )
                tpg = psum_pool.tile([P, COLS], f32, tag="psA", bufs=2,
                                     name=f"tpg{m}")
                nc.tensor.transpose(tpg[:], m2g[:], ident[:COLS, :COLS])
                glr = singles.tile([P, CHUNKS], f32, name=f"glr{m}")
                nc.vector.tensor_copy(out=glr[:], in_=tpg[:])
                glrs.append(glr)

                # wrap-16 scatter index layout: ix[q, 8a+r] = slot[128a+16r+q]
                # = m2s[a, 16r+q]; per r: PE-transpose m2s[:, 16r:16r+16]
                ix = singles.tile([P, F // 16], i16, name=f"idx{m}")
                ixv = ix[0:16, :].rearrange("q (a r) -> q a r", r=8)
                for r in range(8):
                    tpr = psum_pool.tile([16, COLS], f32, tag="psA", bufs=2,
                                         name=f"tpr{m}_{r}")
                    nc.tensor.transpose(
                        tpr[:], m2s[:, 16 * r : 16 * (r + 1)],
                        ident[:COLS, :COLS],
                    )
                    nc.vector.tensor_copy(out=ixv[:, :, r], in_=tpr[:])
                nc.sync.dma_start(out=ix[16:32, :], in_=ix[0:16, :])
                nc.sync.dma_start(out=ix[32:64, :], in_=ix[0:32, :])
                nc.sync.dma_start(out=ix[64:128, :], in_=ix[0:64, :])
                ix16.append(ix)

            # ---------- fwd: PE-transpose fe + SWDGE scatter ----------
            # (the DMA xbar transpose races its consumers under Tile, so use
            # the PE transpose path; fe is already bf16)
            def fwd_prep(m, bb):
                f0 = bb * BF
                src = src_pool.tile([P, BCHUNK, SRC_W], bf16, tag="rows")
                for h in range(2):
                    fet = src_pool.tile([P, BF], bf16, tag="fet")
                    (nc.sync if h == 0 else nc.scalar).dma_start(
                        out=fet[:],
                        in_=fe_d[m * C + h * P : m * C + (h + 1) * P,
                                 f0 : f0 + BF],
                    )
                    a = 0
                    while a < BCHUNK:
                        g = min(4, BCHUNK - a)
                        ps = psum_pool.tile([P, 4 * P], bf16, tag="ps",
                                            bufs=2)
                        for k in range(g):
                            nc.tensor.transpose(
                                ps[:, k * P : (k + 1) * P],
                                fet[:, (a + k) * P : (a + k + 1) * P],
                                identb[:],
                            )
                        eng = nc.vector if (a // 4) % 2 == 0 else nc.scalar
                        if eng is nc.vector:
                            eng.tensor_copy(
                                out=src[:, a : a + g, h * P : (h + 1) * P],
                                in_=ps[:, : g * P].rearrange(
                                    "p (a c) -> p a c", a=g),
                            )
                        else:
                            eng.copy(
                                out=src[:, a : a + g, h * P : (h + 1) * P],
                                in_=ps[:, : g * P].rearrange(
                                    "p (a c) -> p a c", a=g),
                            )
                        a += g
                nc.vector.memset(src[:, :, 256:257], 1.0)
                nc.vector.tensor_copy(
                    out=src[:, :, 257:258],
                    in_=glrs[m][:, bb * BCHUNK : (bb + 1) * BCHUNK, None],
                )
                nc.vector.memset(src[:, :, 258:SRC_W], 0.0)
                return src

            def fwd_scatter(m, bb, src):
                nc.gpsimd.dma_scatter_add(
                    tbl_ds[m][:, 0:SRC_W],
                    src[:],
                    ix16[m][:, bb * (BF // 16) : (bb + 1) * (BF // 16)],
                    BF,
                    BF,
                    SRC_W,
                    elem_step=EW,
                    single_packet=True,
                )

            # ---------- phase 2: per-bucket one-hot matmul + output ----------
            def bwd(m, b):
                tb = tb_pool.tile([P, NCH, SRC_W], bf16, tag="p2rows")
                nc.scalar.dma_start(
                    out=tb[:],
                    in_=bass.AP(
                        tensor=tbl_ds[m],
                        offset=b * CAP * EW,
                        ap=[[EW, P], [P * EW, NCH], [1, SRC_W]],
                    ),
                )
                glc = p2_pool.tile([P, NCH], f32, tag="glc")
                nc.vector.tensor_copy(out=glc[:], in_=tb[:, :, 257])
                pts = [
                    psum_pool.tile([P, 258], f32, tag="ps2", bufs=3,
                                   name=f"pt{m}_{b}_{j}")
                    for j in range(2)
                ]
                for ch in range(NCH):
                    oh = p2_pool.tile([P, BW], bf16, tag="oh")
                    nc.vector.tensor_scalar(oh[:], iob[:],
                                            glc[:, ch : ch + 1],
                                            None, AL.is_equal)
                    for j in range(2):
                        nc.tensor.matmul(
                            pts[j][:],
                            oh[:, j * P : (j + 1) * P],
                            tb[:, ch, 0:258],
                            start=(ch == 0),
                            stop=(ch == NCH - 1),
                        )
                sb2 = p2_pool.tile([P, 2, BW], bf16, tag="sb2", bufs=3)
                for j in range(2):
                    rcp = p2_pool.tile([P, 1], f32, tag="rcp")
                    nc.vector.tensor_scalar(rcp[:], pts[j][:, 256:257],
                                            1.0, None, AL.max)
                    nc.vector.reciprocal(out=rcp[:], in_=rcp[:])
                    if (b + j) % 2:
                        nc.scalar.mul(sb2[:, j, :], pts[j][:, 0:256],
                                      rcp[:, 0:1])
                    else:
                        nc.vector.tensor_scalar(sb2[:, j, :],
                                                pts[j][:, 0:256],
                                                rcp[:, 0:1], None, AL.mult)
                for j in range(2):
                    tw = min(P, TPB - j * P)
                    nc.sync.dma_start(
                        out=bass.AP(
                            tensor=out_d,
                            offset=(m * T + b * TPB + j * P) * C,
                            ap=[[C, tw], [1, C]],
                        ),
                        in_=sb2[:tw, j, :],
                    )

            phaseA(0)
            phaseA(1)
            # prefetch the first two batches of each mesh ahead of the
            # zero DMAs (loads + transposes only — scatters follow the zeros)
            pre = [fwd_prep(0, 0), fwd_prep(1, 0),
                   fwd_prep(0, 1), fwd_prep(1, 1)]
            # zero the tables (contiguous, full rows; scatter is +=) — after
            # phase A/prefetch so their DMAs aren't queued behind 31 MB
            zcols = SLOTS * EW // P
            for m in range(MPC):
                for i, k in enumerate(range(0, zcols, 4096)):
                    w = min(4096, zcols - k)
                    (nc.sync if i % 2 == 0 else nc.scalar).dma_start(
                        out=bass.AP(tensor=tbl_ds[m], offset=k,
                                    ap=[[zcols, P], [1, w]]),
                        in_=zeros[:, :w],
                    )
            fwd_scatter(0, 0, pre[0])
            fwd_scatter(1, 0, pre[1])
            fwd_scatter(0, 1, pre[2])
            fwd_scatter(1, 1, pre[3])
            for bb in range(2, NBATCH):
                for m in range(MPC):
                    fwd_scatter(m, bb, fwd_prep(m, bb))
            for m in range(MPC):
                for b in range(NBUK):
                    bwd(m, b)

    nc.compile()
    return nc


_NC_CACHE = {}


def get_nc():
    if "nc" not in _NC_CACHE:
        _NC_CACHE["nc"] = build_nc()
    return _NC_CACHE["nc"]


TRACE = False
LAST_RESULT = None


def kernel(fe, group_ids):
    global LAST_RESULT
    import ml_dtypes
    from concourse.bass_utils import run_bass_kernel_spmd

    fe = np.asarray(fe)
    if fe.dtype != ml_dtypes.bfloat16:
        fe = fe.astype(np.float32).astype(ml_dtypes.bfloat16)
    gid = np.asarray(group_ids).astype(np.int32)

    nc = get_nc()
    in_maps = []
    for c in range(N_CORES):
        in_maps.append({
            "fe": np.ascontiguousarray(
                fe[c * MPC : (c + 1) * MPC].reshape(MPC * C, F)
            ),
            "gid": np.ascontiguousarray(gid[c * MPC : (c + 1) * MPC]),
        })
    res = run_bass_kernel_spmd(
        nc, in_maps, list(range(N_CORES)), trace=TRACE
    )
    LAST_RESULT = res
    out = np.empty((B, C, T), dtype=np.float32)
    for c in range(N_CORES):
        o = res.results[c]["out"][: MPC * T].astype(np.float32).reshape(
            MPC, T, C)
        out[c * MPC : (c + 1) * MPC] = o.transpose(0, 2, 1)
    return out


# revision 43
# speedup vs baseline: 1.0799x; 1.0799x over previous
"""MeshPoolFace segment-mean pooling kernel for Trainium2 (8 NeuronCores).

Problem: fe [B=16, C=256, F=16000] f32, group_ids [B, F] int in [0, 8000)
Output: [B, C, 8000] f32 = per-(mesh,channel) segment mean of face features.

Data-parallel over B (2 meshes per core). fe is converted to bf16 on the
host (~0.4% quantization, well under the 2e-2 gate); the device writes the
output bf16 in [T, C] layout and the host transposes/upcasts.

Per mesh, on device:
  phase A: bucket = gid // 250 (32 uniform buckets x 250 targets). Compute
    a unique, collision-free scatter slot per face: slot = bucket*CAP +
    rank-within-bucket via matmul prefix sums (strict-lower-triangular
    matmuls). Collision-free slots make the scatter's CCE add an exact
    row write (no read-modify-write races between SDMA engines).
  fwd: stream fe face-major via the DMA xbar transpose (no PE work),
    assemble rows [256 data | 1.0 | gl+1 | pad] (bf16, 260 payload) and
    dma_scatter_add them into the binned DRAM table [32*CAP, 384]
    (row stride 384 elems = 768 B, multiple of 256 B as SWDGE requires;
    only 260 elems per row are written/read). Padding slots (rows
    [416, CAP) of each bucket; every bucket count is >= 443 for this
    input distribution) are pre-zeroed by 4 strided broadcast DMAs.
  phase 2: per bucket, load its CAP rows (260-wide), build one-hot
    (gl+1 vs iota 1..256) in bf16, matmul-accumulate sums+counts in
    PSUM, divide via Scalar-engine per-partition scale, and DMA the
    [targets, C] rows straight out (no transpose on device).
"""

import sys

sys.path.insert(0, "/opt/trn_rl_repo")

import numpy as np

B, C, F, T = 16, 256, 16000, 8000
N_CORES = 8
MPC = B // N_CORES  # meshes per core

P = 128
NBUK = 32            # buckets (gid // 250)
TPB = 250            # targets per bucket (uniform: 32*250 = 8000)
CAP = 640            # slots per bucket (max observed count 562)
NCH = CAP // P       # 5 chunks per bucket
SLOTS = NBUK * CAP   # 20480
EW = 384             # table row stride (bf16 elems; 768 B = 3*256 B)
SRC_W = 260          # scattered payload: 256 data | 1.0 | gl+1 | 2 pad
BW = 256             # one-hot width (gl+1 in [1,250])
# (full-table zeroing; scatter is +=)

NBATCH = 5
CHUNKS = F // P            # 125
BCHUNK = CHUNKS // NBATCH  # 25 chunks per scatter batch
BF = BCHUNK * P            # 3200 faces per batch
COLS = F // P              # 125 (phase-A compute layout columns)


def build_nc():
    import concourse.bacc as bacc
    import concourse.bass as bass
    import concourse.tile as tile
    from concourse import library_config, mybir
    from concourse.masks import make_identity

    f32 = mybir.dt.float32
    bf16 = mybir.dt.bfloat16
    i32 = mybir.dt.int32
    i16 = mybir.dt.int16
    AL = mybir.AluOpType

    nc = bacc.Bacc("TRN2", debug=False)

    fe_d = nc.dram_tensor("fe", [MPC * C, F], bf16, kind="ExternalInput")
    gid_d = nc.dram_tensor("gid", [MPC, F], i32, kind="ExternalInput")
    out_d = nc.dram_tensor("out", [MPC * T + 8, C], bf16,
                           kind="ExternalOutput")
    tbl_ds = [
        nc.dram_tensor(f"tbl{m}", [SLOTS, EW], bf16) for m in range(MPC)
    ]
    slot_d = nc.dram_tensor("slot_s", [MPC, F], f32)
    gl_d = nc.dram_tensor("gl_s", [MPC, F], f32)
    b2_d = nc.dram_tensor("b2_s", [MPC, COLS, NBUK], f32)


    with tile.TileContext(nc) as tc:
        with (
            tc.tile_pool(name="singles", bufs=1) as singles,
            tc.tile_pool(name="masks", bufs=1) as mask_pool,
            tc.tile_pool(name="b2p", bufs=1) as b2_pool,
            tc.tile_pool(name="small", bufs=4) as small_pool,
            tc.tile_pool(name="src", bufs=4) as src_pool,
            tc.tile_pool(name="tbp", bufs=4) as tb_pool,
            tc.tile_pool(name="p2", bufs=6) as p2_pool,
            tc.tile_pool(name="psum", bufs=1, space="PSUM") as psum_pool,
        ):
            nc.gpsimd.load_library(library_config.mlp)

            # ---------- constants ----------
            ident = singles.tile([P, P], f32)
            make_identity(nc, ident[:])
            identb = singles.tile([P, P], bf16)
            nc.vector.tensor_copy(out=identb[:], in_=ident[:])

            ones_col = singles.tile([P, 1], f32)
            nc.vector.memset(ones_col[:], 1.0)
            ones_row = singles.tile([1, COLS], f32)
            nc.vector.memset(ones_row[:], 1.0)

            # strict lower triangular: L[p, x] = 1 iff p < x
            Ls = singles.tile([P, P], f32)
            nc.gpsimd.memset(Ls[:], 0.0)
            nc.gpsimd.affine_select(
                out=Ls[:], in_=Ls[:], pattern=[[-1, P]],
                compare_op=AL.is_ge, fill=1.0, base=0, channel_multiplier=1,
            )
            # augmented [COLS+1, COLS]: strict-lower + all-ones last row
            La = singles.tile([COLS + 1, COLS], f32)
            nc.gpsimd.memset(La[:], 0.0)
            nc.gpsimd.affine_select(
                out=La[:], in_=La[:], pattern=[[-1, COLS]],
                compare_op=AL.is_ge, fill=1.0, base=0, channel_multiplier=1,
            )
            nc.sync.dma_start(out=La[COLS : COLS + 1, :], in_=ones_row[:])

            # iota row 1..256 (bf16, exact) for one-hot compare
            io32 = singles.tile([P, BW], i32)
            nc.gpsimd.iota(io32[:], pattern=[[1, BW]], base=1,
                           channel_multiplier=0)
            iob = singles.tile([P, BW], bf16)
            nc.vector.tensor_copy(out=iob[:], in_=io32[:])

            # row of bucket bases b*CAP
            bc32 = singles.tile([1, NBUK], i32)
            nc.gpsimd.iota(bc32[:], pattern=[[CAP, NBUK]], base=0,
                           channel_multiplier=0)
            bcap = singles.tile([1, NBUK], f32)
            nc.vector.tensor_copy(out=bcap[:], in_=bc32[:])

            zeros = singles.tile([P, 4096], bf16)
            nc.gpsimd.memset(zeros[:], 0.0)

            INV = float(np.float32(1.0) / np.float32(TPB))

            glrs, ix16 = [], []

            # ---------- phase A ----------
            def phaseA(m):
                g32 = small_pool.tile([P, COLS], i32, tag="g32",
                                      name=f"g32_{m}")
                nc.sync.dma_start(out=g32[:], in_=gid_d[m, :].rearrange(
                    "(p c) -> p c", p=P))
                gf = small_pool.tile([P, COLS], f32, tag="gf",
                                     name=f"gf_{m}")
                nc.vector.tensor_copy(out=gf[:], in_=g32[:])
                # bucket = floor(g/250), robust to either f32->int rounding
                xb = small_pool.tile([P, COLS], f32, tag="xb")
                nc.vector.tensor_scalar(xb[:], gf[:], INV, None, AL.mult)
                y32 = small_pool.tile([P, COLS], i32, tag="y32")
                nc.vector.tensor_copy(out=y32[:], in_=xb[:])
                bkf = small_pool.tile([P, COLS], f32, tag="bkf",
                                      name=f"bkf_{m}")
                nc.vector.tensor_copy(out=bkf[:], in_=y32[:])
                over = small_pool.tile([P, COLS], f32, tag="over")
                nc.vector.tensor_tensor(over[:], bkf[:], xb[:], AL.is_gt)
                nc.vector.tensor_tensor(bkf[:], bkf[:], over[:], AL.subtract)
                # gl+1 = g - 250*bucket + 1
                glf = small_pool.tile([P, COLS], f32, tag="glf",
                                      name=f"glf_{m}")
                nc.vector.tensor_scalar(glf[:], bkf[:], float(-TPB), 1.0,
                                        AL.mult, AL.add)
                nc.vector.tensor_tensor(glf[:], glf[:], gf[:], AL.add)
                # gl+1 to DRAM (face-order f = p*COLS + c)
                nc.scalar.dma_start(
                    out=bass.AP(tensor=gl_d, offset=m * F,
                                ap=[[COLS, P], [1, COLS]]),
                    in_=glf[:],
                )

                # bucket masks
                M = mask_pool.tile([P, NBUK, COLS], f32, tag="M")
                for b in range(NBUK):
                    nc.vector.tensor_scalar(M[:, b, :], bkf[:], float(b),
                                            None, AL.is_equal)

                # per-column bucket counts -> [COLS, NBUK] psum
                cnt_ps = psum_pool.tile([COLS, NBUK], f32, tag="psA", bufs=2)
                for b in range(NBUK):
                    nc.tensor.matmul(cnt_ps[:, b : b + 1], M[:, b, :],
                                     ones_col[:], start=True, stop=True)
                cnt_aug = small_pool.tile([COLS + 1, NBUK], f32, tag="caug")
                nc.vector.tensor_copy(out=cnt_aug[0:COLS, :], in_=cnt_ps[:])
                nc.sync.dma_start(out=cnt_aug[COLS : COLS + 1, :],
                                  in_=bcap[:])
                # base2'[c, b] = sum_{c'<c} cnt[c', b] + b*CAP
                b2_ps = psum_pool.tile([COLS, NBUK], f32, tag="psA", bufs=2)
                nc.tensor.matmul(b2_ps[:], La[:], cnt_aug[:],
                                 start=True, stop=True)
                b2_sb = small_pool.tile([COLS, NBUK], f32, tag="b2sb")
                nc.vector.tensor_copy(out=b2_sb[:], in_=b2_ps[:])
                nc.sync.dma_start(out=b2_d[m, :, :], in_=b2_sb[:])
                # broadcast bases to all partitions
                B2 = b2_pool.tile([P, COLS * NBUK], f32, tag="B2")
                nc.scalar.dma_start(
                    out=B2[:],
                    in_=bass.AP(tensor=b2_d, offset=m * COLS * NBUK,
                                ap=[[0, P], [1, COLS * NBUK]]),
                )
                B2v = B2[:].rearrange("p (c b) -> p c b", b=NBUK)

                # slot[p,c] = rank-within-bucket + base2'[c,b] for own bucket
                NACC = 4
                accs = [
                    small_pool.tile([P, COLS], f32, tag=f"acc{a}",
                                    name=f"acc{m}_{a}")
                    for a in range(NACC)
                ]
                NQ = 4  # buckets per cs matmul
                for q in range(NBUK // NQ):
                    cs_ps = psum_pool.tile([P, NQ * COLS], f32, tag="psB",
                                           bufs=1, name=f"cs{m}_{q}")
                    nc.tensor.matmul(
                        cs_ps[:],
                        Ls[:],
                        M[:, q * NQ : (q + 1) * NQ, :].rearrange(
                            "p b c -> p (b c)"),
                        start=True, stop=True,
                    )
                    cs_sb = small_pool.tile([P, NQ * COLS], f32, tag="cssb",
                                            name=f"cssb{m}_{q}")
                    nc.scalar.copy(out=cs_sb[:], in_=cs_ps[:])
                    for bi in range(NQ):
                        b = q * NQ + bi
                        eng = nc.vector
                        acc = accs[(b % 2) * 2 + (b // 2) % 2]
                        t1 = small_pool.tile([P, COLS], f32, tag="t1",
                                             name=f"t1_{m}_{b}")
                        eng.tensor_tensor(t1[:],
                                          cs_sb[:, bi * COLS : (bi + 1) * COLS],
                                          B2v[:, :, b], AL.add)
                        if b < NACC:
                            eng.tensor_tensor(acc[:], t1[:], M[:, b, :],
                                              AL.mult)
                        else:
                            t2 = small_pool.tile([P, COLS], f32, tag="t2",
                                                 name=f"t2_{m}_{b}")
                            eng.tensor_tensor(t2[:], t1[:], M[:, b, :],
                                              AL.mult)
                            eng.tensor_tensor(acc[:], acc[:], t2[:], AL.add)
                slotf = small_pool.tile([P, COLS], f32, tag="slotf")
                nc.vector.tensor_tensor(accs[0][:], accs[0][:], accs[2][:],
                                        AL.add)
                nc.vector.tensor_tensor(accs[1][:], accs[1][:], accs[3][:],
                                        AL.add)
                nc.vector.tensor_tensor(slotf[:], accs[0][:], accs[1][:],
                                        AL.add)
                nc.sync.dma_start(
                    out=bass.AP(tensor=slot_d, offset=m * F,
                                ap=[[COLS, P], [1, COLS]]),
                    in_=slotf[:],
                )

                # reload slot/gl as [125, 128] (contiguous, f = 128a + x);
                # gl: PE-transpose to face-chunk layout [x, a]
                m2s = small_pool.tile([COLS, P], f32, tag="m2s")
                nc.scalar.dma_start(
                    out=m2s[:],
                    in_=slot_d[m, :].rearrange("(a x) -> a x", a=COLS),
                )
                m2g = small_pool.tile([COLS, P], f32, tag="m2g")
                nc.scalar.dma_start(
                    out=m2g[:],
                    in_=gl_d[m, :].rearrange("(a x) -> a x", a=COLS),
                )
                tpg = psum_pool.tile([P, COLS], f32, tag="psA", bufs=2,
                                     name=f"tpg{m}")
                nc.tensor.transpose(tpg[:], m2g[:], ident[:COLS, :COLS])
                glr = singles.tile([P, CHUNKS], f32, name=f"glr{m}")
                nc.vector.tensor_copy(out=glr[:], in_=tpg[:])
                glrs.append(glr)

                # wrap-16 scatter index layout: ix[q, 8a+r] = slot[128a+16r+q]
                # = m2s[a, 16r+q]; per r: PE-transpose m2s[:, 16r:16r+16]
                ix = singles.tile([P, F // 16], i16, name=f"idx{m}")
                ixv = ix[0:16, :].rearrange("q (a r) -> q a r", r=8)
                for r in range(8):
                    tpr = psum_pool.tile([16, COLS], f32, tag="psA", bufs=2,
                                         name=f"tpr{m}_{r}")
                    nc.tensor.transpose(
                        tpr[:], m2s[:, 16 * r : 16 * (r + 1)],
                        ident[:COLS, :COLS],
                    )
                    nc.vector.tensor_copy(out=ixv[:, :, r], in_=tpr[:])
                nc.sync.dma_start(out=ix[16:32, :], in_=ix[0:16, :])
                nc.sync.dma_start(out=ix[32:64, :], in_=ix[0:32, :])
                nc.sync.dma_start(out=ix[64:128, :], in_=ix[0:64, :])
                ix16.append(ix)

            # ---------- fwd: PE-transpose fe + SWDGE scatter ----------
            # (the DMA xbar transpose races its consumers under Tile, so use
            # the PE transpose path; fe is already bf16)
            def fwd_prep(m, bb):
                f0 = bb * BF
                src = src_pool.tile([P, BCHUNK, SRC_W], bf16, tag="rows")
                for h in range(2):
                    fet = src_pool.tile([P, BF], bf16, tag="fet")
                    (nc.sync if h == 0 else nc.scalar).dma_start(
                        out=fet[:],
                        in_=fe_d[m * C + h * P : m * C + (h + 1) * P,
                                 f0 : f0 + BF],
                    )
                    a = 0
                    while a < BCHUNK:
                        g = min(4, BCHUNK - a)
                        ps = psum_pool.tile([P, 4 * P], bf16, tag="ps",
                                            bufs=2)
                        for k in range(g):
                            nc.tensor.transpose(
                                ps[:, k * P : (k + 1) * P],
                                fet[:, (a + k) * P : (a + k + 1) * P],
                                identb[:],
                            )
                        eng = nc.vector if (a // 4) % 2 == 0 else nc.scalar
                        if eng is nc.vector:
                            eng.tensor_copy(
                                out=src[:, a : a + g, h * P : (h + 1) * P],
                                in_=ps[:, : g * P].rearrange(
                                    "p (a c) -> p a c", a=g),
                            )
                        else:
                            eng.copy(
                                out=src[:, a : a + g, h * P : (h + 1) * P],
                                in_=ps[:, : g * P].rearrange(
                                    "p (a c) -> p a c", a=g),
                            )
                        a += g
                nc.vector.memset(src[:, :, 256:257], 1.0)
                nc.vector.tensor_copy(
                    out=src[:, :, 257:258],
                    in_=glrs[m][:, bb * BCHUNK : (bb + 1) * BCHUNK, None],
                )
                nc.vector.memset(src[:, :, 258:SRC_W], 0.0)
                return src

            def fwd_scatter(m, bb, src):
                nc.gpsimd.dma_scatter_add(
                    tbl_ds[m][:, 0:SRC_W],
                    src[:],
                    ix16[m][:, bb * (BF // 16) : (bb + 1) * (BF // 16)],
                    BF,
                    BF,
                    SRC_W,
                    elem_step=EW,
                    single_packet=False,
                )

            # ---------- phase 2: per-bucket one-hot matmul + output ----------
            def bwd(m, b):
                tb = tb_pool.tile([P, NCH, SRC_W], bf16, tag="p2rows")
                nc.scalar.dma_start(
                    out=tb[:],
                    in_=bass.AP(
                        tensor=tbl_ds[m],
                        offset=b * CAP * EW,
                        ap=[[EW, P], [P * EW, NCH], [1, SRC_W]],
                    ),
                )
                glc = p2_pool.tile([P, NCH], f32, tag="glc")
                nc.vector.tensor_copy(out=glc[:], in_=tb[:, :, 257])
                pts = [
                    psum_pool.tile([P, 258], f32, tag="ps2", bufs=3,
                                   name=f"pt{m}_{b}_{j}")
                    for j in range(2)
                ]
                for ch in range(NCH):
                    oh = p2_pool.tile([P, BW], bf16, tag="oh")
                    nc.vector.tensor_scalar(oh[:], iob[:],
                                            glc[:, ch : ch + 1],
                                            None, AL.is_equal)
                    for j in range(2):
                        nc.tensor.matmul(
                            pts[j][:],
                            oh[:, j * P : (j + 1) * P],
                            tb[:, ch, 0:258],
                            start=(ch == 0),
                            stop=(ch == NCH - 1),
                        )
                sb2 = p2_pool.tile([P, 2, BW], bf16, tag="sb2", bufs=3)
                for j in range(2):
                    rcp = p2_pool.tile([P, 1], f32, tag="rcp")
                    nc.vector.tensor_scalar(rcp[:], pts[j][:, 256:257],
                                            1.0, None, AL.max)
                    nc.vector.reciprocal(out=rcp[:], in_=rcp[:])
                    if (b + j) % 2:
                        nc.scalar.mul(sb2[:, j, :], pts[j][:, 0:256],
                                      rcp[:, 0:1])
                    else:
                        nc.vector.tensor_scalar(sb2[:, j, :],
                                                pts[j][:, 0:256],
                                                rcp[:, 0:1], None, AL.mult)
                for j in range(2):
                    tw = min(P, TPB - j * P)
                    nc.sync.dma_start(
                        out=bass.AP(
                            tensor=out_d,
                            offset=(m * T + b * TPB + j * P) * C,
                            ap=[[C, tw], [1, C]],
                        ),
                        in_=sb2[:tw, j, :],
                    )

            phaseA(0)
            phaseA(1)
            # prefetch the first batch of each mesh ahead of the zero DMAs
            # (loads + transposes only — the scatters must follow the zeros)
            pre = [fwd_prep(0, 0), fwd_prep(1, 0)]
            # zero the tables (contiguous, full rows; scatter is +=) — after
            # phase A/prefetch so their DMAs aren't queued behind 31 MB
            zcols = SLOTS * EW // P
            for m in range(MPC):
                for i, k in enumerate(range(0, zcols, 4096)):
                    w = min(4096, zcols - k)
                    (nc.sync if i % 2 == 0 else nc.scalar).dma_start(
                        out=bass.AP(tensor=tbl_ds[m], offset=k,
                                    ap=[[zcols, P], [1, w]]),
                        in_=zeros[:, :w],
                    )
            fwd_scatter(0, 0, pre[0])
            fwd_scatter(1, 0, pre[1])
            for bb in range(1, NBATCH):
                for m in range(MPC):
                    fwd_scatter(m, bb, fwd_prep(m, bb))
            for m in range(MPC):
                for b in range(NBUK):
                    bwd(m, b)

    nc.compile()
    return nc


_NC_CACHE = {}


def get_nc():
    if "nc" not in _NC_CACHE:
        _NC_CACHE["nc"] = build_nc()
    return _NC_CACHE["nc"]


TRACE = False
LAST_RESULT = None


def kernel(fe, group_ids):
    global LAST_RESULT
    import ml_dtypes
    from concourse.bass_utils import run_bass_kernel_spmd

    fe = np.asarray(fe)
    if fe.dtype != ml_dtypes.bfloat16:
        fe = fe.astype(np.float32).astype(ml_dtypes.bfloat16)
    gid = np.asarray(group_ids).astype(np.int32)

    nc = get_nc()
    in_maps = []
    for c in range(N_CORES):
        in_maps.append({
            "fe": np.ascontiguousarray(
                fe[c * MPC : (c + 1) * MPC].reshape(MPC * C, F)
            ),
            "gid": np.ascontiguousarray(gid[c * MPC : (c + 1) * MPC]),
        })
    res = run_bass_kernel_spmd(
        nc, in_maps, list(range(N_CORES)), trace=TRACE
    )
    LAST_RESULT = res
    out = np.empty((B, C, T), dtype=np.float32)
    for c in range(N_CORES):
        o = res.results[c]["out"][: MPC * T].astype(np.float32).reshape(
            MPC, T, C)
        out[c * MPC : (c + 1) * MPC] = o.transpose(0, 2, 1)
    return out
